# revision 8
# baseline (speedup 1.0000x reference)
"""Trainium2 Bass kernel for the deterministic legality module.

Computes, for each board b, filter f and top-left placement (i,j):
    legal[b,f,i,j] = 1.0 iff every occupied cell of filter f, placed at
    (i,j), lands in-bounds on a free cell of board b (and f is non-empty).

Structure exploited (all derived from the actual filter data at
kernel-build time, so the kernel stays correct for any filter set):

  * A placement (i,j) of filter f with max tap offsets (mdy, mdx) is
    structurally illegal unless i <= 8-mdy and j <= 8-mdx; only ~1/3 of
    the 264*81 output columns are reachable.  The device computes ONLY
    those C columns; the host scatters into the zeroed full output.
  * Duplicate filter patterns share one device column.
  * legal = relu(corr + thr) exactly, with the threshold rows folded
    into the contraction (board side carries ones rows).
  * The 0/1 result goes to HBM as int8 (4x less write traffic).

Contraction K = 84: k 0..80 = board cells, k 81/82 = two threshold rows
(thr = 1-area split as ceil/floor halves so every value is fp8
e4m3-exact), k 83 = zero pad.  M ([84, C] fp8) is built ON THE HOST and
DMA'd in; there is no on-device build phase.  fp8 runs at bf16 speed on
the PE but halves the M load bytes.

Sharding: pure data parallelism, batch 4096 -> 512 per core on 8 cores.
"""

import numpy as np
import ml_dtypes

N_CORES = 8
BATCH = 4096
BPC = BATCH // N_CORES  # 512 boards per core
NPOS = 81               # 9x9 board cells / placements
NF = 264                # filters
K = 84                  # 81 cells + 2 thr rows + zero pad

COL_TILE = 512          # one PSUM bank of f32
COL_GROUP = 1024        # columns per drain / output DMA chunk
WARMUP_MM = 4           # dummy matmuls bridging input-DMA latency


def _plan(filters):
    """Host-side column plan + M matrix from the filter data.

    Returns (M[K, c_pad] fp8, c_pad, f_sc, ij_sc, c_sc) where the
    scatter triplet satisfies full[:, f_sc, ij_sc] = raw[:, c_sc].
    """
    filt = np.asarray(filters, dtype=np.float32).reshape(NF, 5, 5)
    areas = filt.sum(axis=(1, 2))
    occ = filt > 0.5

    nonempty = np.where(areas > 0.5)[0]
    keys = (occ.reshape(NF, 25).astype(np.int64)
            * (1 << np.arange(25, dtype=np.int64))).sum(axis=1)
    _, first, inv = np.unique(keys[nonempty], return_index=True,
                              return_inverse=True)
    reps = nonempty[first]          # representative filter per pattern
    U = len(reps)

    mdy = np.array([occ[r].any(axis=1).nonzero()[0].max() for r in reps])
    mdx = np.array([occ[r].any(axis=0).nonzero()[0].max() for r in reps])

    cols = []                       # (u, i, j), ij-major
    col_of = {}                     # (u, ij) -> c
    for i in range(9):
        for j in range(9):
            for u in range(U):
                if mdy[u] <= 8 - i and mdx[u] <= 8 - j:
                    col_of[(u, i * 9 + j)] = len(cols)
                    cols.append((u, i, j))
    C = len(cols)
    c_pad = -(-C // COL_TILE) * COL_TILE

    M = np.zeros((K, c_pad), dtype=np.float32)
    for c, (u, i, j) in enumerate(cols):
        r = reps[u]
        dys, dxs = np.nonzero(occ[r])
        M[(i + dys) * 9 + (j + dxs), c] = 1.0
        thr = 1.0 - areas[r]
        M[NPOS, c] = np.ceil(thr / 2)       # thr_a, in [-12, 0]
        M[NPOS + 1, c] = np.floor(thr / 2)  # thr_b, in [-12, 0]

    f_sc, ij_sc, c_sc = [], [], []
    for fi, f in enumerate(nonempty):
        u = inv[fi]
        for ij in range(NPOS):
            c = col_of.get((u, ij))
            if c is not None:
                f_sc.append(f)
                ij_sc.append(ij)
                c_sc.append(c)
    return (M.astype(ml_dtypes.float8_e4m3fn), c_pad,
            np.asarray(f_sc), np.asarray(ij_sc), np.asarray(c_sc))


def _build_module(c_pad):
    import concourse.bass as bass
    import concourse.mybir as mybir
    import concourse.tile as tile
    from concourse.masks import make_identity

    f32 = mybir.dt.float32
    fp8 = mybir.dt.float8e4
    i8 = mybir.dt.int8
    Relu = mybir.ActivationFunctionType.Relu

    nc = bass.Bass("TRN2", target_bir_lowering=False, debug=False,
                   num_devices=N_CORES)

    board_d = nc.dram_tensor("board", [BPC, NPOS], f32, kind="ExternalInput")
    m_d = nc.dram_tensor("mmat", [K, c_pad], fp8, kind="ExternalInput")
    out_d = nc.dram_tensor("out", [BPC, c_pad], i8, kind="ExternalOutput")

    groups = [(g, min(g + COL_GROUP, c_pad))
              for g in range(0, c_pad, COL_GROUP)]
    nkb = BPC // 128

    with tile.TileContext(nc) as tc:
        with tc.tile_pool(name="const", bufs=1) as cpool:
            ident = cpool.tile([128, 128], f32)
            make_identity(nc, ident[:])

            M = cpool.tile([K, c_pad], fp8)
            boardT = cpool.tile([K, BPC], fp8)

            # ---- phase A: inputs, transposes, act-table preload --------
            with (
                tc.tile_pool(name="prep", bufs=2) as prep,
                tc.tile_pool(name="psA", bufs=2, space="PSUM") as psA,
                tc.tile_pool(name="psW", bufs=1, space="PSUM") as psW,
            ):
                # board (512,81) f32 -> btile rows of 84 cols per board:
                # [cells 0..80, 1.0, 1.0, 0.0]; one 84-col transpose per
                # 128-board block.
                btile = prep.tile([128, 4 * K], f32, tag="bload")
                bt3 = btile[:].rearrange("p (c y) -> p c y", c=4)
                nc.gpsimd.memset(bt3[:, :, NPOS:NPOS + 2], 1.0)
                nc.gpsimd.memset(bt3[:, :, NPOS + 2:K], 0.0)
                bview = board_d[:].rearrange("(c p) x -> p c x", p=128)
                nc.sync.dma_start(bt3[:, :, 0:NPOS], bview)

                # M loads: group 0 split in partition halves; groups 1-2
                # halved across SP/ACT; the rest single DMAs on SP.  ACT's
                # triggers are emitted before its first drains.
                h = K // 2
                for gi, (g0, g1) in enumerate(groups):
                    if gi == 0:
                        nc.sync.dma_start(M[0:h, g0:g1], m_d[0:h, g0:g1])
                        nc.scalar.dma_start(M[h:K, g0:g1], m_d[h:K, g0:g1])
                    elif gi <= 2:
                        nc.sync.dma_start(M[0:h, g0:g1], m_d[0:h, g0:g1])
                        nc.scalar.dma_start(M[h:K, g0:g1], m_d[h:K, g0:g1])
                    else:
                        nc.sync.dma_start(M[:, g0:g1], m_d[:, g0:g1])

                # preload the Relu activation table off the critical path
                warm8 = prep.tile([1, 1], i8, tag="warm8")
                wps0 = psW.tile([128, 128], f32, tag="warm")
                nc.scalar.activation(warm8[:], wps0[0:1, 0:1], Relu)

                if WARMUP_MM:
                    for _ in range(WARMUP_MM):
                        nc.tensor.matmul(wps0[:], ident[:], ident[:],
                                         start=True, stop=True)
                    wrd = prep.tile([32, 1], f32, tag="wrd")
                    nc.vector.tensor_scalar_add(wrd[:], wps0[0:32, 0:1], 0.0)

                for kb in range(nkb):
                    bps = psA.tile([K, 128], f32, tag="btp")
                    nc.tensor.transpose(bps[:], btile[:, kb * K:(kb + 1) * K],
                                        ident[:])
                    nc.vector.tensor_scalar_add(
                        boardT[:, kb * 128:(kb + 1) * 128], bps[:], 0.0)

            # ---- phase B: matmul + relu(int8) + store ------------------
            # group-outer / kb-inner.  Drains split DVE/ACT ~12/16 (ACT
            # is faster per column).  Output chunks: early thirds on the
            # software DGE (keeps SP free), middle on SP, final wave as
            # half-height chunks on SP+ACT for a short tail.
            with (
                tc.tile_pool(name="psM", bufs=4, space="PSUM") as psM,
                tc.tile_pool(name="ostage", bufs=6) as ostage,
            ):
                # drain engine pattern per chunk index (A=Act, V=DVE)
                pat = [1, 0, 1, 1, 0, 1, 0]   # per group of 7, x4 kb
                last_g0 = groups[-1][0]
                ci = 0
                for gi, (g0, g1) in enumerate(groups):
                    gw = g1 - g0
                    for kb in range(nkb):
                        lhsT = boardT[:, kb * 128:(kb + 1) * 128]
                        pt = psM.tile([128, COL_GROUP], f32, tag="mm")
                        for off in range(0, gw, COL_TILE):
                            w = min(COL_TILE, gw - off)
                            nc.tensor.matmul(
                                pt[:, off:off + w], lhsT,
                                M[:, g0 + off:g0 + off + w],
                                start=True, stop=True)
                        ot = ostage.tile([128, COL_GROUP], i8, tag="ot")
                        if pat[gi % 7]:
                            nc.scalar.activation(ot[:, :gw], pt[:, :gw], Relu)
                        else:
                            nc.vector.tensor_scalar_max(
                                ot[:, :gw], pt[:, :gw], 0.0)
                        ci += 1
                        rows = out_d[kb * 128:(kb + 1) * 128, g0:g1]
                        if g0 == last_g0:
                            # final wave: half-height chunks on SP + ACT
                            nc.sync.dma_start(rows[0:64, :], ot[0:64, :gw])
                            nc.scalar.dma_start(rows[64:128, :],
                                                ot[64:128, :gw])
                        elif gi < 3:
                            nc.gpsimd.dma_start(rows, ot[:, :gw])
                        else:
                            nc.sync.dma_start(rows, ot[:, :gw])
    return nc


def _legalize_multiwait(nc):
    """Split multi-wait instructions for this walrus build.

    The TPB instruction encodings carry exactly one semaphore wait, and
    the walrus codegen here refuses instructions with more ("Too many
    sync wait commands").  Hoist all but one wait onto EventSemaphore
    carrier instructions placed immediately before, on the same engine —
    the sequencer blocks on each carrier first, which is semantically
    identical.
    """
    import concourse.mybir as mybir

    for func in nc.m.functions:
        for blk in func.blocks:
            out = []
            changed = False
            for inst in blk.instructions:
                si = inst.sync_info
                waits = list(si.on_wait) if si is not None and si.on_wait else []
                if len(waits) > 1:
                    for j, w in enumerate(waits[:-1]):
                        carrier = mybir.InstEventSemaphore(
                            name=f"{inst.name}-xw{j}",
                            engine=inst.engine,
                            ins=[], outs=[],
                            sync_info=mybir.SyncInfo(on_wait=[w],
                                                     on_update=[]),
                        )
                        nc.register_instruction(carrier)
                        out.append(carrier)
                    inst.sync_info = mybir.SyncInfo(
                        on_wait=[waits[-1]],
                        on_update=list(si.on_update) if si.on_update else [])
                    changed = True
                out.append(inst)
            if changed:
                blk.instructions = out


_CACHE = {}


def _get_module(c_pad):
    if c_pad not in _CACHE:
        nc = _build_module(c_pad)
        _legalize_multiwait(nc)
        _CACHE[c_pad] = nc
    return _CACHE[c_pad]


def run(board_free, filters, areas, trace=False, **spmd_kwargs):
    from concourse.bass_utils import run_bass_kernel_spmd

    M, c_pad, f_sc, ij_sc, c_sc = _plan(filters)

    board = np.ascontiguousarray(
        np.asarray(board_free, dtype=np.float32).reshape(N_CORES, BPC, NPOS))

    in_maps = [
        {"board": board[c], "mmat": M}
        for c in range(N_CORES)
    ]
    nc = _get_module(c_pad)
    res = run_bass_kernel_spmd(nc, in_maps, core_ids=list(range(N_CORES)),
                               trace=trace, **spmd_kwargs)
    raw = np.concatenate([r["out"] for r in res.results], axis=0)

    full = np.zeros((BATCH, NF, NPOS), dtype=np.float32)
    full[:, f_sc, ij_sc] = raw[:, c_sc]
    return full.reshape(BATCH, NF, 9, 9), res


def kernel(board_free, filters, areas):
    out, _ = run(board_free, filters, areas)
    return out


# revision 10
# speedup vs baseline: 1.0006x; 1.0006x over previous
"""Trainium2 Bass kernel for the deterministic legality module.

Computes, for each board b, filter f and top-left placement (i,j):
    legal[b,f,i,j] = 1.0 iff every occupied cell of filter f, placed at
    (i,j), lands in-bounds on a free cell of board b (and f is non-empty).

Structure exploited (all derived from the actual filter data at
kernel-build time, so the kernel stays correct for any filter set):

  * A placement (i,j) of filter f with max tap offsets (mdy, mdx) is
    structurally illegal unless i <= 8-mdy and j <= 8-mdx; only ~1/3 of
    the 264*81 output columns are reachable.  The device computes ONLY
    those C columns; the host scatters into the zeroed full output.
  * Duplicate filter patterns share one device column.
  * legal = relu(corr + thr) exactly, with the threshold rows folded
    into the contraction (board side carries ones rows).
  * The 0/1 result goes to HBM as int8 (4x less write traffic).

The matmul runs in fp8e4 DoubleRow mode: K = 84 as two k-tiles of 42
(k = q*42+p), which streams two contraction rows per cycle -> 0.5
cycles per output column, measured ~213ns per 512-column tile.
k 0..80 = board cells, k 81/82 = two threshold rows (thr = 1-area split
ceil/floor so every value is e4m3-exact), k 83 = zero pad.  M
([42, 2, C] fp8) is built ON THE HOST and DMA'd in; there is no
on-device build phase.

Sharding: pure data parallelism, batch 4096 -> 512 per core on 8 cores.
"""

import numpy as np
import ml_dtypes

N_CORES = 8
BATCH = 4096
BPC = BATCH // N_CORES  # 512 boards per core
NPOS = 81               # 9x9 board cells / placements
NF = 264                # filters
KT = 42                 # k-tile size (DoubleRow)
K = 2 * KT              # logical contraction: 81 cells + 2 thr + pad

COL_TILE = 512          # one PSUM bank of f32
COL_GROUP = 1024        # columns per drain / output DMA chunk
WARMUP_MM = 4           # dummy matmuls bridging input-DMA latency


def _plan(filters):
    """Host-side column plan + DoubleRow M matrix from the filter data.

    Returns (M[KT, 2*c_pad] fp8, c_pad, f_sc, ij_sc, c_sc) where the
    scatter triplet satisfies full[:, f_sc, ij_sc] = raw[:, c_sc].
    M[p, q*c_pad + c] holds logical row k = q*42+p: taps for k<=80,
    thr_a at 81, thr_b at 82, zero at 83.
    """
    filt = np.asarray(filters, dtype=np.float32).reshape(NF, 5, 5)
    areas = filt.sum(axis=(1, 2))
    occ = filt > 0.5

    nonempty = np.where(areas > 0.5)[0]
    keys = (occ.reshape(NF, 25).astype(np.int64)
            * (1 << np.arange(25, dtype=np.int64))).sum(axis=1)
    _, first, inv = np.unique(keys[nonempty], return_index=True,
                              return_inverse=True)
    reps = nonempty[first]          # representative filter per pattern
    U = len(reps)

    mdy = np.array([occ[r].any(axis=1).nonzero()[0].max() for r in reps])
    mdx = np.array([occ[r].any(axis=0).nonzero()[0].max() for r in reps])

    cols = []                       # (u, i, j), ij-major
    col_of = {}                     # (u, ij) -> c
    for i in range(9):
        for j in range(9):
            for u in range(U):
                if mdy[u] <= 8 - i and mdx[u] <= 8 - j:
                    col_of[(u, i * 9 + j)] = len(cols)
                    cols.append((u, i, j))
    C = len(cols)
    c_pad = -(-C // COL_TILE) * COL_TILE

    M82 = np.zeros((K, c_pad), dtype=np.float32)
    for c, (u, i, j) in enumerate(cols):
        r = reps[u]
        dys, dxs = np.nonzero(occ[r])
        M82[(i + dys) * 9 + (j + dxs), c] = 1.0
        thr = 1.0 - areas[r]
        M82[NPOS, c] = np.ceil(thr / 2)       # thr_a, in [-12, 0]
        M82[NPOS + 1, c] = np.floor(thr / 2)  # thr_b, in [-12, 0]

    # interleave into DoubleRow k-tiles: M[p, q, c] = M82[q*42+p, c]
    M = np.ascontiguousarray(
        M82.reshape(2, KT, c_pad).transpose(1, 0, 2).reshape(KT, 2 * c_pad))

    f_sc, ij_sc, c_sc = [], [], []
    for fi, f in enumerate(nonempty):
        u = inv[fi]
        for ij in range(NPOS):
            c = col_of.get((u, ij))
            if c is not None:
                f_sc.append(f)
                ij_sc.append(ij)
                c_sc.append(c)
    return (M.astype(ml_dtypes.float8_e4m3fn), c_pad,
            np.asarray(f_sc), np.asarray(ij_sc), np.asarray(c_sc))


def _build_module(c_pad):
    import concourse.bass as bass
    import concourse.mybir as mybir
    import concourse.tile as tile
    from concourse.masks import make_identity

    f32 = mybir.dt.float32
    fp8 = mybir.dt.float8e4
    i8 = mybir.dt.int8
    Relu = mybir.ActivationFunctionType.Relu
    DR = mybir.MatmulPerfMode.DoubleRow

    nc = bass.Bass("TRN2", target_bir_lowering=False, debug=False,
                   num_devices=N_CORES)

    board_d = nc.dram_tensor("board", [BPC, NPOS], f32, kind="ExternalInput")
    m_d = nc.dram_tensor("mmat", [KT, 2 * c_pad], fp8, kind="ExternalInput")
    out_d = nc.dram_tensor("out", [BPC, c_pad], i8, kind="ExternalOutput")

    groups = [(g, min(g + COL_GROUP, c_pad))
              for g in range(0, c_pad, COL_GROUP)]
    nkb = BPC // 128

    with tile.TileContext(nc) as tc:
        with tc.tile_pool(name="const", bufs=1) as cpool:
            ident = cpool.tile([128, 128], f32)
            make_identity(nc, ident[:])

            M = cpool.tile([KT, 2 * c_pad], fp8)
            M3 = M[:].rearrange("p (q n) -> p q n", q=2)
            m3 = m_d[:].rearrange("p (q n) -> p q n", q=2)
            boardT = cpool.tile([KT, 2 * BPC], fp8)
            bT3 = boardT[:].rearrange("p (q n) -> p q n", q=2)

            # ---- phase A: inputs, transposes, act-table preload --------
            with (
                tc.tile_pool(name="prep", bufs=2) as prep,
                tc.tile_pool(name="psA", bufs=2, space="PSUM") as psA,
                tc.tile_pool(name="psW", bufs=1, space="PSUM") as psW,
            ):
                # board (512,81) f32 -> btile rows of 84 cols per board:
                # [cells 0..80, 1.0, 1.0, 0.0]; two 42-col transposes per
                # 128-board block produce the two k-tiles at partition 0.
                btile = prep.tile([128, 4 * K], f32, tag="bload")
                bt3 = btile[:].rearrange("p (c y) -> p c y", c=4)
                nc.gpsimd.memset(bt3[:, :, NPOS:NPOS + 2], 1.0)
                nc.gpsimd.memset(bt3[:, :, NPOS + 2:K], 0.0)
                bview = board_d[:].rearrange("(c p) x -> p c x", p=128)

                # SP order: g0 halves, board, then the rest of M.
                # ACT gets the b-halves of g1/g2 before its first drain.
                h = KT // 2

                def m_load(eng, p0, p1, g0, g1):
                    eng.dma_start(M3[p0:p1, :, g0:g1], m3[p0:p1, :, g0:g1])

                m_load(nc.sync, 0, h, *groups[0])
                m_load(nc.sync, h, KT, *groups[0])
                nc.sync.dma_start(bt3[:, :, 0:NPOS], bview)
                m_load(nc.scalar, 0, KT, *groups[1])
                m_load(nc.sync, 0, KT, *groups[2])
                for g in groups[3:]:
                    m_load(nc.sync, 0, KT, *g)

                # preload the Relu activation table on an independent
                # dummy read of ident so nothing serializes behind the
                # 1.3us table load
                warm8 = prep.tile([1, 2], i8, tag="warm8")
                nc.scalar.activation(warm8[:], ident[0:1, 0:2], Relu)

                if WARMUP_MM:
                    wps0 = psW.tile([128, 128], f32, tag="warm")
                    for _ in range(WARMUP_MM):
                        nc.tensor.matmul(wps0[:], ident[:], ident[:],
                                         start=True, stop=True)
                    wrd = prep.tile([32, 1], f32, tag="wrd")
                    nc.vector.tensor_scalar_add(wrd[:], wps0[0:32, 0:1], 0.0)

                for kb in range(nkb):
                    bps = psA.tile([KT, 256], f32, tag="btp")
                    for q in range(2):
                        nc.tensor.transpose(
                            bps[:, q * 128:(q + 1) * 128],
                            btile[:, kb * K + q * KT:kb * K + (q + 1) * KT],
                            ident[:])
                    src = bps[:].rearrange("p (q n) -> p q n", q=2)
                    nc.vector.tensor_scalar_add(
                        bT3[:, :, kb * 128:(kb + 1) * 128], src, 0.0)

            # ---- phase B: DoubleRow matmul + relu(int8) + store --------
            # group-outer / kb-inner.  Drains alternate DVE/ACT except
            # the final group (all DVE, so ACT is free to co-trigger the
            # final output wave).  Output chunks: first three groups on
            # the software DGE, middle on SP, final wave as half-height
            # chunks on SP+ACT for a short tail.
            with (
                tc.tile_pool(name="psM", bufs=4, space="PSUM") as psM,
                tc.tile_pool(name="ostage", bufs=6) as ostage,
            ):
                last_g0 = groups[-1][0]
                ci = 0
                for gi, (g0, g1) in enumerate(groups):
                    gw = g1 - g0
                    final = g0 == last_g0
                    for kb in range(nkb):
                        lhsT = bT3[:, :, kb * 128:(kb + 1) * 128]
                        pt = psM.tile([128, COL_GROUP], f32, tag="mm")
                        for off in range(0, gw, COL_TILE):
                            w = min(COL_TILE, gw - off)
                            nc.tensor.matmul(
                                pt[:, off:off + w], lhsT,
                                M3[:, :, g0 + off:g0 + off + w],
                                start=True, stop=True, perf_mode=DR)
                        ot = ostage.tile([128, COL_GROUP], i8, tag="ot")
                        use_act = (ci % 2 == 1) and not final
                        if use_act:
                            nc.scalar.activation(ot[:, :gw], pt[:, :gw], Relu)
                        else:
                            nc.vector.tensor_scalar_max(
                                ot[:, :gw], pt[:, :gw], 0.0)
                        ci += 1
                        rows = out_d[kb * 128:(kb + 1) * 128, g0:g1]
                        if final:
                            nc.sync.dma_start(rows[0:64, :], ot[0:64, :gw])
                            nc.scalar.dma_start(rows[64:128, :],
                                                ot[64:128, :gw])
                        elif gi < 3:
                            nc.gpsimd.dma_start(rows, ot[:, :gw])
                        else:
                            nc.sync.dma_start(rows, ot[:, :gw])
    return nc


def _legalize_multiwait(nc):
    """Split multi-wait instructions for this walrus build.

    The TPB instruction encodings carry exactly one semaphore wait, and
    the walrus codegen here refuses instructions with more ("Too many
    sync wait commands").  Hoist all but one wait onto EventSemaphore
    carrier instructions placed immediately before, on the same engine —
    the sequencer blocks on each carrier first, which is semantically
    identical.
    """
    import concourse.mybir as mybir

    for func in nc.m.functions:
        for blk in func.blocks:
            out = []
            changed = False
            for inst in blk.instructions:
                si = inst.sync_info
                waits = list(si.on_wait) if si is not None and si.on_wait else []
                if len(waits) > 1:
                    for j, w in enumerate(waits[:-1]):
                        carrier = mybir.InstEventSemaphore(
                            name=f"{inst.name}-xw{j}",
                            engine=inst.engine,
                            ins=[], outs=[],
                            sync_info=mybir.SyncInfo(on_wait=[w],
                                                     on_update=[]),
                        )
                        nc.register_instruction(carrier)
                        out.append(carrier)
                    inst.sync_info = mybir.SyncInfo(
                        on_wait=[waits[-1]],
                        on_update=list(si.on_update) if si.on_update else [])
                    changed = True
                out.append(inst)
            if changed:
                blk.instructions = out


_CACHE = {}


def _get_module(c_pad):
    if c_pad not in _CACHE:
        nc = _build_module(c_pad)
        _legalize_multiwait(nc)
        _CACHE[c_pad] = nc
    return _CACHE[c_pad]


def run(board_free, filters, areas, trace=False, **spmd_kwargs):
    from concourse.bass_utils import run_bass_kernel_spmd

    M, c_pad, f_sc, ij_sc, c_sc = _plan(filters)

    board = np.ascontiguousarray(
        np.asarray(board_free, dtype=np.float32).reshape(N_CORES, BPC, NPOS))

    in_maps = [
        {"board": board[c], "mmat": M}
        for c in range(N_CORES)
    ]
    nc = _get_module(c_pad)
    res = run_bass_kernel_spmd(nc, in_maps, core_ids=list(range(N_CORES)),
                               trace=trace, **spmd_kwargs)
    raw = np.concatenate([r["out"] for r in res.results], axis=0)

    full = np.zeros((BATCH, NF, NPOS), dtype=np.float32)
    full[:, f_sc, ij_sc] = raw[:, c_sc]
    return full.reshape(BATCH, NF, 9, 9), res


def kernel(board_free, filters, areas):
    out, _ = run(board_free, filters, areas)
    return out


# revision 12
# speedup vs baseline: 1.0038x; 1.0032x over previous
"""Trainium2 Bass kernel for the deterministic legality module.

Computes, for each board b, filter f and top-left placement (i,j):
    legal[b,f,i,j] = 1.0 iff every occupied cell of filter f, placed at
    (i,j), lands in-bounds on a free cell of board b (and f is non-empty).

Structure exploited (all derived from the actual filter data at
kernel-build time, so the kernel stays correct for any filter set):

  * A placement (i,j) of filter f with max tap offsets (mdy, mdx) is
    structurally illegal unless i <= 8-mdy and j <= 8-mdx; only ~1/3 of
    the 264*81 output columns are reachable.  The device computes ONLY
    those C columns; the host scatters into the zeroed full output.
  * Duplicate filter patterns share one device column.
  * legal = relu(corr + thr) exactly, with the threshold rows folded
    into the contraction (board side carries ones rows).
  * The 0/1 result goes to HBM as int8 (4x less write traffic).

The matmul runs in fp8e4 DoubleRow mode: K = 84 as two k-tiles of 42
(k = q*42+p), which streams two contraction rows per cycle -> 0.5
cycles per output column, measured ~213ns per 512-column tile.
k 0..80 = board cells, k 81/82 = two threshold rows (thr = 1-area split
ceil/floor so every value is e4m3-exact), k 83 = zero pad.  M
([42, 2, C] fp8) is built ON THE HOST and DMA'd in; there is no
on-device build phase.

Sharding: pure data parallelism, batch 4096 -> 512 per core on 8 cores.
"""

import numpy as np
import ml_dtypes

N_CORES = 8
BATCH = 4096
BPC = BATCH // N_CORES  # 512 boards per core
NPOS = 81               # 9x9 board cells / placements
NF = 264                # filters
KT = 42                 # k-tile size (DoubleRow)
K = 2 * KT              # logical contraction: 81 cells + 2 thr + pad

COL_TILE = 512          # one PSUM bank of f32
COL_GROUP = 1024        # columns per drain / output DMA chunk
WARMUP_MM = 4           # dummy matmuls bridging input-DMA latency


def _plan(filters):
    """Host-side column plan + DoubleRow M matrix from the filter data.

    Returns (M[KT, 2*c_pad] fp8, c_pad, f_sc, ij_sc, c_sc) where the
    scatter triplet satisfies full[:, f_sc, ij_sc] = raw[:, c_sc].
    M[p, q*c_pad + c] holds logical row k = q*42+p: taps for k<=80,
    thr_a at 81, thr_b at 82, zero at 83.
    """
    filt = np.asarray(filters, dtype=np.float32).reshape(NF, 5, 5)
    areas = filt.sum(axis=(1, 2))
    occ = filt > 0.5

    nonempty = np.where(areas > 0.5)[0]
    keys = (occ.reshape(NF, 25).astype(np.int64)
            * (1 << np.arange(25, dtype=np.int64))).sum(axis=1)
    _, first, inv = np.unique(keys[nonempty], return_index=True,
                              return_inverse=True)
    reps = nonempty[first]          # representative filter per pattern
    U = len(reps)

    mdy = np.array([occ[r].any(axis=1).nonzero()[0].max() for r in reps])
    mdx = np.array([occ[r].any(axis=0).nonzero()[0].max() for r in reps])

    cols = []                       # (u, i, j), ij-major
    col_of = {}                     # (u, ij) -> c
    for i in range(9):
        for j in range(9):
            for u in range(U):
                if mdy[u] <= 8 - i and mdx[u] <= 8 - j:
                    col_of[(u, i * 9 + j)] = len(cols)
                    cols.append((u, i, j))
    C = len(cols)
    c_pad = -(-C // COL_TILE) * COL_TILE

    M82 = np.zeros((K, c_pad), dtype=np.float32)
    for c, (u, i, j) in enumerate(cols):
        r = reps[u]
        dys, dxs = np.nonzero(occ[r])
        M82[(i + dys) * 9 + (j + dxs), c] = 1.0
        thr = 1.0 - areas[r]
        M82[NPOS, c] = np.ceil(thr / 2)       # thr_a, in [-12, 0]
        M82[NPOS + 1, c] = np.floor(thr / 2)  # thr_b, in [-12, 0]

    # interleave into DoubleRow k-tiles: M[p, q, c] = M82[q*42+p, c]
    M = np.ascontiguousarray(
        M82.reshape(2, KT, c_pad).transpose(1, 0, 2).reshape(KT, 2 * c_pad))

    f_sc, ij_sc, c_sc = [], [], []
    for fi, f in enumerate(nonempty):
        u = inv[fi]
        for ij in range(NPOS):
            c = col_of.get((u, ij))
            if c is not None:
                f_sc.append(f)
                ij_sc.append(ij)
                c_sc.append(c)
    return (M.astype(ml_dtypes.float8_e4m3fn), c_pad,
            np.asarray(f_sc), np.asarray(ij_sc), np.asarray(c_sc))


def _build_module(c_pad):
    import concourse.bass as bass
    import concourse.mybir as mybir
    import concourse.tile as tile
    from concourse.masks import make_identity

    f32 = mybir.dt.float32
    fp8 = mybir.dt.float8e4
    i8 = mybir.dt.int8
    Relu = mybir.ActivationFunctionType.Relu
    DR = mybir.MatmulPerfMode.DoubleRow

    nc = bass.Bass("TRN2", target_bir_lowering=False, debug=False,
                   num_devices=N_CORES)

    board_d = nc.dram_tensor("board", [BPC, NPOS], f32, kind="ExternalInput")
    m_d = nc.dram_tensor("mmat", [KT, 2 * c_pad], fp8, kind="ExternalInput")
    out_d = nc.dram_tensor("out", [BPC, c_pad], i8, kind="ExternalOutput")

    groups = [(g, min(g + COL_GROUP, c_pad))
              for g in range(0, c_pad, COL_GROUP)]
    nkb = BPC // 128

    with tile.TileContext(nc) as tc:
        with tc.tile_pool(name="const", bufs=1) as cpool:
            ident = cpool.tile([128, 128], f32)
            make_identity(nc, ident[:])

            M = cpool.tile([KT, 2 * c_pad], fp8)
            M3 = M[:].rearrange("p (q n) -> p q n", q=2)
            m3 = m_d[:].rearrange("p (q n) -> p q n", q=2)
            boardT = cpool.tile([KT, 2 * BPC], fp8)
            bT3 = boardT[:].rearrange("p (q n) -> p q n", q=2)

            # ---- phase A: inputs, transposes, act-table preload --------
            with (
                tc.tile_pool(name="prep", bufs=2) as prep,
                tc.tile_pool(name="psA", bufs=2, space="PSUM") as psA,
                tc.tile_pool(name="psW", bufs=1, space="PSUM") as psW,
            ):
                # board (512,81) f32 -> btile rows of 84 cols per board:
                # [cells 0..80, 1.0, 1.0, 0.0]; two 42-col transposes per
                # 128-board block produce the two k-tiles at partition 0.
                btile = prep.tile([128, 4 * K], f32, tag="bload")
                bt3 = btile[:].rearrange("p (c y) -> p c y", c=4)
                nc.gpsimd.memset(bt3[:, :, NPOS:NPOS + 2], 1.0)
                nc.gpsimd.memset(bt3[:, :, NPOS + 2:K], 0.0)
                bview = board_d[:].rearrange("(c p) x -> p c x", p=128)

                # SP order: g0 halves, board, then the rest of M.
                # ACT gets the b-halves of g1/g2 before its first drain.
                h = KT // 2

                def m_load(eng, p0, p1, g0, g1):
                    eng.dma_start(M3[p0:p1, :, g0:g1], m3[p0:p1, :, g0:g1])

                m_load(nc.sync, 0, h, *groups[0])
                m_load(nc.sync, h, KT, *groups[0])
                nc.sync.dma_start(bt3[:, :, 0:NPOS], bview)
                m_load(nc.sync, 0, h, *groups[1])
                m_load(nc.scalar, h, KT, *groups[1])
                m_load(nc.sync, 0, h, *groups[2])
                m_load(nc.scalar, h, KT, *groups[2])
                for g in groups[3:]:
                    m_load(nc.sync, 0, KT, *g)

                # preload the Relu activation table on an independent
                # dummy read of ident so nothing serializes behind the
                # 1.3us table load
                warm8 = prep.tile([1, 2], i8, tag="warm8")
                nc.scalar.activation(warm8[:], ident[0:1, 0:2], Relu)

                if WARMUP_MM:
                    wps0 = psW.tile([128, 128], f32, tag="warm")
                    for _ in range(WARMUP_MM):
                        nc.tensor.matmul(wps0[:], ident[:], ident[:],
                                         start=True, stop=True)
                    wrd = prep.tile([32, 1], f32, tag="wrd")
                    nc.vector.tensor_scalar_add(wrd[:], wps0[0:32, 0:1], 0.0)

                for kb in range(nkb):
                    bps = psA.tile([KT, 256], f32, tag="btp")
                    for q in range(2):
                        nc.tensor.transpose(
                            bps[:, q * 128:(q + 1) * 128],
                            btile[:, kb * K + q * KT:kb * K + (q + 1) * KT],
                            ident[:])
                    src = bps[:].rearrange("p (q n) -> p q n", q=2)
                    nc.vector.tensor_scalar_add(
                        bT3[:, :, kb * 128:(kb + 1) * 128], src, 0.0)

            # ---- phase B: DoubleRow matmul + relu(int8) + store --------
            # group-outer / kb-inner.  Drains alternate DVE/ACT except
            # the final group (all DVE, so ACT is free to co-trigger the
            # final output wave).  Output chunks: first three groups on
            # the software DGE, middle on SP, final wave as half-height
            # chunks on SP+ACT for a short tail.
            nchunks = len(groups) * nkb
            with (
                tc.tile_pool(name="psM", bufs=4, space="PSUM") as psM,
                tc.tile_pool(name="ostage", bufs=nchunks) as ostage,
            ):
                # ostage has one buffer per chunk: no reuse, so drains
                # never wait on output DMAs and the DMA triggers can lag
                # the compute freely.
                last_g0 = groups[-1][0]
                ci = 0
                for gi, (g0, g1) in enumerate(groups):
                    gw = g1 - g0
                    final = g0 == last_g0
                    for kb in range(nkb):
                        lhsT = bT3[:, :, kb * 128:(kb + 1) * 128]
                        pt = psM.tile([128, COL_GROUP], f32, tag="mm")
                        for off in range(0, gw, COL_TILE):
                            w = min(COL_TILE, gw - off)
                            nc.tensor.matmul(
                                pt[:, off:off + w], lhsT,
                                M3[:, :, g0 + off:g0 + off + w],
                                start=True, stop=True, perf_mode=DR)
                        ot = ostage.tile([128, COL_GROUP], i8, tag="ot")
                        if ci % 2 == 1:
                            nc.scalar.activation(ot[:, :gw], pt[:, :gw], Relu)
                        else:
                            nc.vector.tensor_scalar_max(
                                ot[:, :gw], pt[:, :gw], 0.0)
                        ci += 1
                        rows = out_d[kb * 128:(kb + 1) * 128, g0:g1]
                        if final:
                            nc.sync.dma_start(rows[0:64, :], ot[0:64, :gw])
                            nc.scalar.dma_start(rows[64:128, :],
                                                ot[64:128, :gw])
                        elif gi < 3:
                            nc.gpsimd.dma_start(rows, ot[:, :gw])
                        else:
                            nc.sync.dma_start(rows, ot[:, :gw])
    return nc


def _legalize_multiwait(nc):
    """Split multi-wait instructions for this walrus build.

    The TPB instruction encodings carry exactly one semaphore wait, and
    the walrus codegen here refuses instructions with more ("Too many
    sync wait commands").  Hoist all but one wait onto EventSemaphore
    carrier instructions placed immediately before, on the same engine —
    the sequencer blocks on each carrier first, which is semantically
    identical.
    """
    import concourse.mybir as mybir

    for func in nc.m.functions:
        for blk in func.blocks:
            out = []
            changed = False
            for inst in blk.instructions:
                si = inst.sync_info
                waits = list(si.on_wait) if si is not None and si.on_wait else []
                if len(waits) > 1:
                    for j, w in enumerate(waits[:-1]):
                        carrier = mybir.InstEventSemaphore(
                            name=f"{inst.name}-xw{j}",
                            engine=inst.engine,
                            ins=[], outs=[],
                            sync_info=mybir.SyncInfo(on_wait=[w],
                                                     on_update=[]),
                        )
                        nc.register_instruction(carrier)
                        out.append(carrier)
                    inst.sync_info = mybir.SyncInfo(
                        on_wait=[waits[-1]],
                        on_update=list(si.on_update) if si.on_update else [])
                    changed = True
                out.append(inst)
            if changed:
                blk.instructions = out


_CACHE = {}


def _get_module(c_pad):
    if c_pad not in _CACHE:
        nc = _build_module(c_pad)
        _legalize_multiwait(nc)
        _CACHE[c_pad] = nc
    return _CACHE[c_pad]


def run(board_free, filters, areas, trace=False, **spmd_kwargs):
    from concourse.bass_utils import run_bass_kernel_spmd

    M, c_pad, f_sc, ij_sc, c_sc = _plan(filters)

    board = np.ascontiguousarray(
        np.asarray(board_free, dtype=np.float32).reshape(N_CORES, BPC, NPOS))

    in_maps = [
        {"board": board[c], "mmat": M}
        for c in range(N_CORES)
    ]
    nc = _get_module(c_pad)
    res = run_bass_kernel_spmd(nc, in_maps, core_ids=list(range(N_CORES)),
                               trace=trace, **spmd_kwargs)
    raw = np.concatenate([r["out"] for r in res.results], axis=0)

    full = np.zeros((BATCH, NF, NPOS), dtype=np.float32)
    full[:, f_sc, ij_sc] = raw[:, c_sc]
    return full.reshape(BATCH, NF, 9, 9), res


def kernel(board_free, filters, areas):
    out, _ = run(board_free, filters, areas)
    return out


# revision 17
# speedup vs baseline: 1.0918x; 1.0876x over previous
"""Trainium2 Bass kernel for the deterministic legality module.

Computes, for each board b, filter f and top-left placement (i,j):
    legal[b,f,i,j] = 1.0 iff every occupied cell of filter f, placed at
    (i,j), lands in-bounds on a free cell of board b (and f is non-empty).

Structure exploited (all derived from the actual filter data at
kernel-build time, so the kernel stays correct for any filter set):

  * A placement (i,j) of filter f with max tap offsets (mdy, mdx) is
    structurally illegal unless i <= 8-mdy and j <= 8-mdx; only ~1/3 of
    the 264*81 output columns are reachable.  The device computes ONLY
    those C columns; the host scatters into the zeroed full output.
  * Duplicate filter patterns share one device column.
  * legal = relu(corr + thr) exactly, with the threshold rows folded
    into the contraction (board side carries ones rows).
  * The 0/1 result goes to HBM as int8 (4x less write traffic).

The matmul runs in fp8e4 DoubleRow mode: K = 84 as two k-tiles of 42
(k = q*42+p), which streams two contraction rows per cycle -> 0.5
cycles per output column, measured ~213ns per 512-column tile.
k 0..80 = board cells, k 81/82 = two threshold rows (thr = 1-area split
ceil/floor so every value is e4m3-exact), k 83 = zero pad.  M
([42, 2, C] fp8) is built ON THE HOST and DMA'd in; there is no
on-device build phase.

Sharding: pure data parallelism, batch 4096 -> 512 per core on 8 cores.
"""

import numpy as np
import ml_dtypes

N_CORES = 8
BATCH = 4096
BPC = BATCH // N_CORES  # 512 boards per core
NPOS = 81               # 9x9 board cells / placements
NF = 264                # filters
KT = 42                 # k-tile size (DoubleRow)
K = 2 * KT              # logical contraction: 81 cells + 2 thr + pad

COL_TILE = 512          # one PSUM bank of f32
COL_GROUP = 1024        # columns per drain / output DMA chunk
WARMUP_MM = 4           # dummy matmuls bridging input-DMA latency


def _plan(filters):
    """Host-side column plan + DoubleRow M matrix from the filter data.

    Returns (M[KT, 2*c_pad] fp8, c_pad, f_sc, ij_sc, c_sc) where the
    scatter triplet satisfies full[:, f_sc, ij_sc] = raw[:, c_sc].
    M[p, q*c_pad + c] holds logical row k = q*42+p: taps for k<=80,
    thr_a at 81, thr_b at 82, zero at 83.
    """
    filt = np.asarray(filters, dtype=np.float32).reshape(NF, 5, 5)
    areas = filt.sum(axis=(1, 2))
    occ = filt > 0.5

    nonempty = np.where(areas > 0.5)[0]
    keys = (occ.reshape(NF, 25).astype(np.int64)
            * (1 << np.arange(25, dtype=np.int64))).sum(axis=1)
    _, first, inv = np.unique(keys[nonempty], return_index=True,
                              return_inverse=True)
    reps = nonempty[first]          # representative filter per pattern
    U = len(reps)

    mdy = np.array([occ[r].any(axis=1).nonzero()[0].max() for r in reps])
    mdx = np.array([occ[r].any(axis=0).nonzero()[0].max() for r in reps])

    cols = []                       # (u, i, j), ij-major
    col_of = {}                     # (u, ij) -> c
    for i in range(9):
        for j in range(9):
            for u in range(U):
                if mdy[u] <= 8 - i and mdx[u] <= 8 - j:
                    col_of[(u, i * 9 + j)] = len(cols)
                    cols.append((u, i, j))
    C = len(cols)
    c_pad = -(-C // COL_GROUP) * COL_GROUP   # even # of 512-tiles (A/B split)

    M82 = np.zeros((K, c_pad), dtype=np.float32)
    for c, (u, i, j) in enumerate(cols):
        r = reps[u]
        dys, dxs = np.nonzero(occ[r])
        M82[(i + dys) * 9 + (j + dxs), c] = 1.0
        thr = 1.0 - areas[r]
        M82[NPOS, c] = np.ceil(thr / 2)       # thr_a, in [-12, 0]
        M82[NPOS + 1, c] = np.floor(thr / 2)  # thr_b, in [-12, 0]

    # interleave into DoubleRow k-tiles: M3[p, q, c] = M82[q*42+p, c],
    # then split columns: even 512-tiles -> A (PE row strips 0-1), odd
    # 512-tiles -> B (strips 2-3) so consecutive matmuls alternate
    # row-groups and their weight loads hide under each other.
    M3 = M82.reshape(2, KT, c_pad).transpose(1, 0, 2)   # [KT, 2, c_pad]
    t = M3.reshape(KT, 2, c_pad // COL_TILE, COL_TILE)
    MA = np.ascontiguousarray(t[:, :, 0::2, :].reshape(KT, -1))
    MB = np.ascontiguousarray(t[:, :, 1::2, :].reshape(KT, -1))

    f_sc, ij_sc, c_sc = [], [], []
    for fi, f in enumerate(nonempty):
        u = inv[fi]
        for ij in range(NPOS):
            c = col_of.get((u, ij))
            if c is not None:
                f_sc.append(f)
                ij_sc.append(ij)
                c_sc.append(c)
    return (MA.astype(ml_dtypes.float8_e4m3fn),
            MB.astype(ml_dtypes.float8_e4m3fn), c_pad,
            np.asarray(f_sc), np.asarray(ij_sc), np.asarray(c_sc))


def _build_module(c_pad):
    import concourse.bass as bass
    import concourse.mybir as mybir
    import concourse.tile as tile
    from concourse.masks import make_identity

    f32 = mybir.dt.float32
    fp8 = mybir.dt.float8e4
    i8 = mybir.dt.int8
    Relu = mybir.ActivationFunctionType.Relu
    DR = mybir.MatmulPerfMode.DoubleRow

    nc = bass.Bass("TRN2", target_bir_lowering=False, debug=False,
                   num_devices=N_CORES)

    board_d = nc.dram_tensor("board", [BPC, NPOS], f32, kind="ExternalInput")
    c_half = c_pad // 2
    ma_d = nc.dram_tensor("mmatA", [KT, 2 * c_half], fp8, kind="ExternalInput")
    mb_d = nc.dram_tensor("mmatB", [KT, 2 * c_half], fp8, kind="ExternalInput")
    out_d = nc.dram_tensor("out", [BPC, c_pad], i8, kind="ExternalOutput")

    groups = [(g, min(g + COL_GROUP, c_pad))
              for g in range(0, c_pad, COL_GROUP)]
    nkb = BPC // 128
    PB = 64                    # partition base of the B row-strip copy

    with tile.TileContext(nc) as tc:
        with tc.tile_pool(name="const", bufs=1) as cpool:
            ident = cpool.tile([128, 128], f32)
            make_identity(nc, ident[:])

            # A copies live at partitions 0..41 (PE row strips 0-1), B
            # copies at 64..105 (strips 2-3): consecutive matmuls
            # alternate strips so LDWEIGHTS overlaps the other strip's
            # matmul instead of serializing with it.
            Mbig = cpool.tile([PB + KT, 2 * c_half], fp8)
            MA3 = Mbig[0:KT, :].rearrange("p (q n) -> p q n", q=2)
            MB3 = Mbig[PB:PB + KT, :].rearrange("p (q n) -> p q n", q=2)
            ma3 = ma_d[:].rearrange("p (q n) -> p q n", q=2)
            mb3 = mb_d[:].rearrange("p (q n) -> p q n", q=2)
            bbig = cpool.tile([PB + KT, 2 * BPC], fp8)
            bA3 = bbig[0:KT, :].rearrange("p (q n) -> p q n", q=2)
            bB3 = bbig[PB:PB + KT, :].rearrange("p (q n) -> p q n", q=2)

            # ---- phase A: inputs, transposes, act-table preload --------
            with (
                tc.tile_pool(name="prep", bufs=2) as prep,
                tc.tile_pool(name="psA", bufs=2, space="PSUM") as psA,
                tc.tile_pool(name="psW", bufs=1, space="PSUM") as psW,
            ):
                # board (512,81) f32 -> btile rows of 84 cols per board:
                # [cells 0..80, 1.0, 1.0, 0.0]; two 42-col transposes per
                # 128-board block produce the two k-tiles at partition 0.
                btile = prep.tile([128, 4 * K], f32, tag="bload")
                bt3 = btile[:].rearrange("p (c y) -> p c y", c=4)
                nc.gpsimd.memset(bt3[:, :, NPOS:NPOS + 2], 1.0)
                nc.gpsimd.memset(bt3[:, :, NPOS + 2:K], 0.0)
                bview = board_d[:].rearrange("(c p) x -> p c x", p=128)

                # per-group M chunks are [KT, 2, 512] (43KB).  SP carries
                # g0 A+B, the board, and the late chunks; ACT takes g1/g2
                # A-halves before its first drain.
                def m_load(eng, which, gi):
                    s0, s1 = gi * COL_TILE, (gi + 1) * COL_TILE
                    if which == 0:
                        eng.dma_start(MA3[:, :, s0:s1], ma3[:, :, s0:s1])
                    else:
                        eng.dma_start(MB3[:, :, s0:s1], mb3[:, :, s0:s1])

                ngr = len(groups)
                m_load(nc.sync, 0, 0)
                m_load(nc.sync, 1, 0)
                nc.sync.dma_start(bt3[:, :, 0:NPOS], bview)
                m_load(nc.scalar, 0, 1)
                m_load(nc.scalar, 0, 2)
                m_load(nc.sync, 1, 1)
                m_load(nc.sync, 1, 2)
                for gi in range(3, ngr):
                    m_load(nc.sync, 0, gi)
                    m_load(nc.sync, 1, gi)

                # preload the Relu activation table on an independent
                # dummy read of ident so nothing serializes behind the
                # 1.3us table load
                warm8 = prep.tile([1, 2], i8, tag="warm8")
                nc.scalar.activation(warm8[:], ident[0:1, 0:2], Relu)

                if WARMUP_MM:
                    wps0 = psW.tile([128, 128], f32, tag="warm")
                    for _ in range(WARMUP_MM):
                        nc.tensor.matmul(wps0[:], ident[:], ident[:],
                                         start=True, stop=True)
                    wrd = prep.tile([32, 1], f32, tag="wrd")
                    nc.vector.tensor_scalar_add(wrd[:], wps0[0:32, 0:1], 0.0)

                for kb in range(nkb):
                    bps = psA.tile([KT, 256], f32, tag="btp")
                    for q in range(2):
                        nc.tensor.transpose(
                            bps[:, q * 128:(q + 1) * 128],
                            btile[:, kb * K + q * KT:kb * K + (q + 1) * KT],
                            ident[:])
                    src = bps[:].rearrange("p (q n) -> p q n", q=2)
                    nc.vector.tensor_scalar_add(
                        bA3[:, :, kb * 128:(kb + 1) * 128], src, 0.0)
                # B copy of boardT via a partition-shifting DMA (ACT,
                # after its two M triggers)
                nc.scalar.dma_start(bbig[PB:PB + KT, :], bbig[0:KT, :])

            # ---- phase B: DoubleRow matmul + relu(int8) + store --------
            # group-outer / kb-inner.  Drains alternate DVE/ACT except
            # the final group (all DVE, so ACT is free to co-trigger the
            # final output wave).  Output chunks: first three groups on
            # the software DGE, middle on SP, final wave as half-height
            # chunks on SP+ACT for a short tail.
            nchunks = len(groups) * nkb
            with (
                tc.tile_pool(name="psM", bufs=4, space="PSUM") as psM,
                tc.tile_pool(name="ostage", bufs=nchunks) as ostage,
            ):
                # ostage has one buffer per chunk: no reuse, so drains
                # never wait on output DMAs and the DMA triggers can lag
                # the compute freely.
                last_g0 = groups[-1][0]
                ci = 0
                for gi, (g0, g1) in enumerate(groups):
                    gw = g1 - g0
                    final = g0 == last_g0
                    for kb in range(nkb):
                        ks = slice(kb * 128, (kb + 1) * 128)
                        ss = slice(gi * COL_TILE, (gi + 1) * COL_TILE)
                        pt = psM.tile([128, COL_GROUP], f32, tag="mm")
                        # even 512-tile from strip A, odd from strip B;
                        # their LDWEIGHTS overlap each other's matmuls
                        nc.tensor.matmul(
                            pt[:, 0:COL_TILE], bA3[:, :, ks],
                            MA3[:, :, ss],
                            start=True, stop=True, perf_mode=DR,
                            tile_position=(0, 0))
                        nc.tensor.matmul(
                            pt[:, COL_TILE:COL_GROUP], bB3[:, :, ks],
                            MB3[:, :, ss],
                            start=True, stop=True, perf_mode=DR,
                            tile_position=(PB, 0))
                        ot = ostage.tile([128, COL_GROUP], i8, tag="ot")
                        if ci % 2 == 1:
                            nc.scalar.activation(ot[:, :gw], pt[:, :gw], Relu)
                        else:
                            nc.vector.tensor_scalar_max(
                                ot[:, :gw], pt[:, :gw], 0.0)
                        ci += 1
                        rows = out_d[kb * 128:(kb + 1) * 128, g0:g1]
                        if final:
                            nc.sync.dma_start(rows[0:64, :], ot[0:64, :gw])
                            nc.scalar.dma_start(rows[64:128, :],
                                                ot[64:128, :gw])
                        elif gi < 3:
                            nc.gpsimd.dma_start(rows, ot[:, :gw])
                        else:
                            nc.sync.dma_start(rows, ot[:, :gw])
    return nc


def _legalize_multiwait(nc):
    """Split multi-wait instructions for this walrus build.

    The TPB instruction encodings carry exactly one semaphore wait, and
    the walrus codegen here refuses instructions with more ("Too many
    sync wait commands").  Hoist all but one wait onto EventSemaphore
    carrier instructions placed immediately before, on the same engine —
    the sequencer blocks on each carrier first, which is semantically
    identical.
    """
    import concourse.mybir as mybir

    for func in nc.m.functions:
        for blk in func.blocks:
            out = []
            changed = False
            for inst in blk.instructions:
                si = inst.sync_info
                waits = list(si.on_wait) if si is not None and si.on_wait else []
                if len(waits) > 1:
                    for j, w in enumerate(waits[:-1]):
                        carrier = mybir.InstEventSemaphore(
                            name=f"{inst.name}-xw{j}",
                            engine=inst.engine,
                            ins=[], outs=[],
                            sync_info=mybir.SyncInfo(on_wait=[w],
                                                     on_update=[]),
                        )
                        nc.register_instruction(carrier)
                        out.append(carrier)
                    inst.sync_info = mybir.SyncInfo(
                        on_wait=[waits[-1]],
                        on_update=list(si.on_update) if si.on_update else [])
                    changed = True
                out.append(inst)
            if changed:
                blk.instructions = out


_CACHE = {}


def _get_module(c_pad):
    if c_pad not in _CACHE:
        nc = _build_module(c_pad)
        _legalize_multiwait(nc)
        _CACHE[c_pad] = nc
    return _CACHE[c_pad]


def run(board_free, filters, areas, trace=False, **spmd_kwargs):
    from concourse.bass_utils import run_bass_kernel_spmd

    MA, MB, c_pad, f_sc, ij_sc, c_sc = _plan(filters)

    board = np.ascontiguousarray(
        np.asarray(board_free, dtype=np.float32).reshape(N_CORES, BPC, NPOS))

    in_maps = [
        {"board": board[c], "mmatA": MA, "mmatB": MB}
        for c in range(N_CORES)
    ]
    nc = _get_module(c_pad)
    res = run_bass_kernel_spmd(nc, in_maps, core_ids=list(range(N_CORES)),
                               trace=trace, **spmd_kwargs)
    raw = np.concatenate([r["out"] for r in res.results], axis=0)

    full = np.zeros((BATCH, NF, NPOS), dtype=np.float32)
    full[:, f_sc, ij_sc] = raw[:, c_sc]
    return full.reshape(BATCH, NF, 9, 9), res


def kernel(board_free, filters, areas):
    out, _ = run(board_free, filters, areas)
    return out
